# revision 4
# baseline (speedup 1.0000x reference)
"""Bass/Tile TRN2 kernel for a non-local attention block (BaseNonLocalBlock).

Contract: kernel(**inputs) takes the FULL inputs of the nn.Module problem
(B=1, D=256, H=4, N=4096) and returns the FULL output [1, 256, 4096].

Sharding: query columns of the N x N attention are split across the 8
NeuronCores (512 queries per core). K/V projections are computed
redundantly on every core (cheap); each core produces its own output
column slice and the host concatenates.

Per-core algorithm (flash-attention style, scores never hit HBM):
  Q = (Wq/8) @ xq + bq/8              [256, 512]   (1/sqrt(DH) folded in)
  K = Wk @ x + bk                     [256, 4096]
  V_T = x^T @ Wv^T (+ones col/head)   [4096, 4*65] (denominator trick)
  per key-chunk i (32 x 128 keys), per head h:
    S_T = K_h[:, i]^T @ Q_h           [128, 512]  (PSUM)
    E = exp(spatialT[i] * S_T)        [128, 512]  (DVE mult, ACT exp)
    msg_h += V_T_aug[i, h]^T @ E      [65, 512]   (PSUM accum; row 64 = denom)
  msg = msg_h[0:64] / msg_h[64]  (reciprocal + DMA partition-broadcast)
  out = xq + W3 @ relu(bn2(W2 @ relu(bn1(W1 @ msg))))   (BN folded into W/b)

All matmuls run as float32r (full-rate fp32 stream on the PE).
"""

import numpy as np
from contextlib import ExitStack

D = 256
N = 4096
NQ = 512          # queries per core
H = 4
DH = 64
NCORES = 8
NIT = N // 128    # 32 key chunks

_CACHE = {}


def _build(has_bv, has_b3):
    import concourse.bass as bass
    import concourse.tile as tile
    from concourse import bacc, mybir

    F32 = mybir.dt.float32
    F32R = mybir.dt.float32r
    Id = mybir.ActivationFunctionType.Identity
    Exp = mybir.ActivationFunctionType.Exp
    Relu = mybir.ActivationFunctionType.Relu

    nc = bacc.Bacc("TRN2", target_bir_lowering=False, debug=False,
                   num_devices=NCORES)

    # DRAM I/O (per core)
    x_d = nc.dram_tensor("x", [D, N], F32R, kind="ExternalInput").ap()
    xq_d = nc.dram_tensor("xq", [D, NQ], F32R, kind="ExternalInput").ap()
    spt_d = nc.dram_tensor("spt", [N, NQ], F32, kind="ExternalInput").ap()
    wqt_d = nc.dram_tensor("wqt", [D, D], F32R, kind="ExternalInput").ap()
    wkt_d = nc.dram_tensor("wkt", [D, D], F32R, kind="ExternalInput").ap()
    wvt_d = nc.dram_tensor("wvt", [D, D], F32R, kind="ExternalInput").ap()
    w1t_d = nc.dram_tensor("w1t", [D, 128], F32R, kind="ExternalInput").ap()
    w2t_d = nc.dram_tensor("w2t", [128, 128], F32R, kind="ExternalInput").ap()
    w3t_d = nc.dram_tensor("w3t", [128, D], F32R, kind="ExternalInput").ap()
    bq_d = nc.dram_tensor("bq2", [128, 2], F32, kind="ExternalInput").ap()
    bk_d = nc.dram_tensor("bk2", [128, 2], F32, kind="ExternalInput").ap()
    bv_d = nc.dram_tensor("bv2", [128, 2], F32, kind="ExternalInput").ap()
    b1_d = nc.dram_tensor("b1f", [128, 1], F32, kind="ExternalInput").ap()
    b2_d = nc.dram_tensor("b2f", [128, 1], F32, kind="ExternalInput").ap()
    b3_d = nc.dram_tensor("b32", [128, 2], F32, kind="ExternalInput").ap()
    out_d = nc.dram_tensor("out", [D, NQ], F32, kind="ExternalOutput").ap()

    spt_t3 = spt_d.rearrange("(t p) o -> t p o", p=128)

    with tile.TileContext(nc) as tc, ExitStack() as ctx:
        sb = ctx.enter_context(tc.tile_pool(name="sb", bufs=1))
        spt_pool = ctx.enter_context(tc.tile_pool(name="sptp", bufs=4))
        e_pool = ctx.enter_context(tc.tile_pool(name="ep", bufs=4))
        ps_t = ctx.enter_context(tc.tile_pool(name="pst", bufs=3, space="PSUM"))
        ps_m = ctx.enter_context(tc.tile_pool(name="psm", bufs=1, space="PSUM"))
        dramp = ctx.enter_context(tc.tile_pool(name="dramp", bufs=1, space="DRAM"))

        # ---- persistent SBUF loads ----
        # x as 2 (row-chunk) x 8 (column-block) tiles for fine-grained deps
        xcb = [[sb.tile([128, 512], F32R, name=f"x{ci}_{ib}") for ib in range(8)]
               for ci in range(2)]
        for ci in range(2):
            for ib in range(8):
                nc.sync.dma_start(xcb[ci][ib][:],
                                  x_d[ci * 128:(ci + 1) * 128,
                                      ib * 512:(ib + 1) * 512])
        xq = [sb.tile([128, NQ], F32R, name=f"xq{co}") for co in range(2)]
        for co in range(2):
            nc.sync.dma_start(xq[co][:], xq_d[co * 128:(co + 1) * 128, :])

        wqt = [sb.tile([128, D], F32R, name=f"wqt{ci}") for ci in range(2)]
        wkt = [sb.tile([128, D], F32R, name=f"wkt{ci}") for ci in range(2)]
        wvt = [sb.tile([128, D], F32R, name=f"wvt{ci}") for ci in range(2)]
        w1t = [sb.tile([128, 128], F32R, name=f"w1t{ci}") for ci in range(2)]
        for ci in range(2):
            sl = slice(ci * 128, (ci + 1) * 128)
            nc.sync.dma_start(wqt[ci][:], wqt_d[sl, :])
            nc.sync.dma_start(wkt[ci][:], wkt_d[sl, :])
            nc.sync.dma_start(wvt[ci][:], wvt_d[sl, :])
            nc.sync.dma_start(w1t[ci][:], w1t_d[sl, :])
        w2t = sb.tile([128, 128], F32R, name="w2t")
        nc.sync.dma_start(w2t[:], w2t_d[:, :])
        w3t = sb.tile([128, D], F32R, name="w3t")
        nc.sync.dma_start(w3t[:], w3t_d[:, :])

        bq = sb.tile([128, 2], F32, name="bq")
        bk = sb.tile([128, 2], F32, name="bk")
        b1 = sb.tile([128, 1], F32, name="b1")
        b2 = sb.tile([128, 1], F32, name="b2")
        nc.sync.dma_start(bq[:], bq_d[:, :])
        nc.sync.dma_start(bk[:], bk_d[:, :])
        nc.sync.dma_start(b1[:], b1_d[:, :])
        nc.sync.dma_start(b2[:], b2_d[:, :])
        if has_bv:
            bv = sb.tile([128, 2], F32, name="bv")
            nc.sync.dma_start(bv[:], bv_d[:, :])
        if has_b3:
            b3 = sb.tile([128, 2], F32, name="b3")
            nc.sync.dma_start(b3[:], b3_d[:, :])

        k_sb = [sb.tile([128, N], F32R, name=f"k{co}") for co in range(2)]
        q_sb = [sb.tile([128, NQ], F32R, name=f"q{co}") for co in range(2)]
        # V^T augmented: per key-chunk it, per head h: [64 V cols | ones]
        vt = sb.tile([128, NIT, H, 66], F32R, name="vt")
        nc.gpsimd.memset(vt[:, :, :, 64:66].bitcast(F32), 1.0)
        msg = [sb.tile([128, NQ], F32R, name=f"msg{co}") for co in range(2)]

        # ---- Q projection ----
        for co in range(2):
            ps = ps_t.tile([128, NQ], F32, tag="t")
            for ci in range(2):
                nc.tensor.matmul(ps[:],
                                 wqt[ci][:, co * 128:(co + 1) * 128],
                                 xq[ci][:],
                                 start=(ci == 0), stop=(ci == 1))
            nc.scalar.activation(q_sb[co][:], ps[:], Id, bias=bq[:, co:co + 1])

        # message accumulators (row 64 = softmax denominator)
        mps = [ps_m.tile([65, NQ], F32, name=f"mps{h}") for h in range(H)]

        # ---- main streaming loop over key chunks ----
        for ib in range(8):
            # K projection for this 512-wide block
            for co in range(2):
                ps = ps_t.tile([128, NQ], F32, tag="t")
                for ci in range(2):
                    nc.tensor.matmul(ps[:],
                                     wkt[ci][:, co * 128:(co + 1) * 128],
                                     xcb[ci][ib][:],
                                     start=(ci == 0), stop=(ci == 1))
                nc.scalar.activation(k_sb[co][:, ib * 512:(ib + 1) * 512],
                                     ps[:], Id, bias=bk[:, co:co + 1])
            for it in range(ib * 4, ib * 4 + 4):
                icol = slice((it % 4) * 128, (it % 4) * 128 + 128)
                # V^T projection for this key chunk
                vps = ps_t.tile([128, D], F32, tag="t")
                for ci in range(2):
                    nc.tensor.matmul(vps[:],
                                     xcb[ci][ib][:, icol],
                                     wvt[ci][:],
                                     start=(ci == 0), stop=(ci == 1))
                nc.scalar.activation(vt[:, it, :, 0:64],
                                     vps.rearrange("p (h c) -> p h c", h=H),
                                     Id)
                spt_t = spt_pool.tile([128, NQ], F32)
                nc.sync.dma_start(spt_t[:], spt_t3[it])
                for h in range(H):
                    co, ro = h // 2, (h % 2) * 64
                    sps = ps_t.tile([128, NQ], F32, tag="t")
                    nc.tensor.matmul(
                        sps[:],
                        k_sb[co][ro:ro + 64, it * 128:(it + 1) * 128],
                        q_sb[co][ro:ro + 64, :],
                        start=True, stop=True)
                    e = e_pool.tile([128, NQ], F32R)
                    nc.vector.tensor_mul(e[:], sps[:], spt_t[:])
                    nc.scalar.activation(e[:], e[:], Exp)
                    nc.tensor.matmul(mps[h][:], vt[:, it, h, 0:65], e[:],
                                     start=(it == 0), stop=(it == NIT - 1))

        # ---- softmax normalization ----
        d4 = sb.tile([4, NQ], F32, name="d4")
        dh_t = []
        for h in range(H):
            dh = sb.tile([1, NQ], F32, name=f"dh{h}")
            nc.scalar.copy(dh[:], mps[h][64:65, :])
            nc.sync.dma_start(d4[h:h + 1, :], dh[:])
            dh_t.append(dh)
        r4 = sb.tile([4, NQ], F32, name="r4")
        scr = sb.tile([4, NQ], F32, name="scr")
        nc.vector.reciprocal_approx_accurate(out=r4[:], in_=d4[:], scratch=scr[:])
        rd = dramp.tile([4, NQ], F32, name="rd")
        nc.sync.dma_start(rd[:], r4[:])
        for h in range(H):
            co, ro = h // 2, (h % 2) * 64
            rbc = sb.tile([64, NQ], F32, name=f"rbc{h}")
            row = rd[h:h + 1, :]
            bsrc = bass.AP(tensor=rd.tensor, offset=row.offset,
                           ap=[[0, 64]] + [list(x) for x in row.ap[1:]])
            nc.sync.dma_start(rbc[:], bsrc)
            nc.vector.tensor_mul(msg[co][ro:ro + 64, :], mps[h][0:64, :], rbc[:])
            if has_bv:
                nc.scalar.activation(msg[co][ro:ro + 64, :],
                                     msg[co][ro:ro + 64, :], Id,
                                     bias=bv[ro:ro + 64, co:co + 1])

        # ---- message MLP + residual ----
        u1 = ps_t.tile([128, NQ], F32, tag="t")
        for ci in range(2):
            nc.tensor.matmul(u1[:], w1t[ci][:], msg[ci][:],
                             start=(ci == 0), stop=(ci == 1))
        h1 = sb.tile([128, NQ], F32R, name="h1")
        nc.scalar.activation(h1[:], u1[:], Relu, bias=b1[:, 0:1])
        u2 = ps_t.tile([128, NQ], F32, tag="t")
        nc.tensor.matmul(u2[:], w2t[:], h1[:], start=True, stop=True)
        h2 = sb.tile([128, NQ], F32R, name="h2")
        nc.scalar.activation(h2[:], u2[:], Relu, bias=b2[:, 0:1])
        for co in range(2):
            u3 = ps_t.tile([128, NQ], F32, tag="t")
            nc.tensor.matmul(u3[:], w3t[:, co * 128:(co + 1) * 128],
                             h2[:], start=True, stop=True)
            ot = sb.tile([128, NQ], F32, name=f"ot{co}")
            if has_b3:
                tb = sb.tile([128, NQ], F32, name=f"tb{co}")
                nc.scalar.activation(tb[:], u3[:], Id, bias=b3[:, co:co + 1])
                nc.vector.tensor_add(ot[:], tb[:], xq[co][:].bitcast(F32))
            else:
                nc.vector.tensor_add(ot[:], u3[:], xq[co][:].bitcast(F32))
            nc.sync.dma_start(out_d[co * 128:(co + 1) * 128, :], ot[:])

    nc.compile()
    return nc


def _prep_inputs(inputs):
    f = lambda a: np.ascontiguousarray(np.asarray(a, dtype=np.float32))
    x = f(inputs["corr_feat_belief"][0])                    # [D, N]
    spT = f(np.asarray(inputs["spatial_compatibility"][0]).T)  # [N(keys), N(queries)]
    Wq, bq = f(inputs["Wq"]), f(inputs["bq"])
    Wk, bk = f(inputs["Wk"]), f(inputs["bk"])
    Wv, bv = f(inputs["Wv"]), f(inputs["bv"])
    W1, b1, g1, be1 = f(inputs["W1"]), f(inputs["b1"]), f(inputs["g1"]), f(inputs["be1"])
    W2, b2, g2, be2 = f(inputs["W2"]), f(inputs["b2"]), f(inputs["g2"]), f(inputs["be2"])
    W3, b3 = f(inputs["W3"]), f(inputs["b3"])

    scale = np.float32(1.0 / np.sqrt(DH))
    s1 = (g1 / np.sqrt(np.float32(1.0) + np.float32(1e-5))).astype(np.float32)
    s2 = (g2 / np.sqrt(np.float32(1.0) + np.float32(1e-5))).astype(np.float32)

    common = dict(
        x=x,
        wqt=f(Wq.T * scale),
        wkt=f(Wk.T),
        wvt=f(Wv.T),
        w1t=f((W1 * s1[:, None]).T),
        w2t=f((W2 * s2[:, None]).T),
        w3t=f(W3.T),
        bq2=f((bq * scale).reshape(2, 128).T),
        bk2=f(bk.reshape(2, 128).T),
        bv2=f(bv.reshape(2, 128).T),
        b1f=f((s1 * b1 + be1).reshape(128, 1)),
        b2f=f((s2 * b2 + be2).reshape(128, 1)),
        b32=f(b3.reshape(2, 128).T),
    )
    in_maps = []
    for m in range(NCORES):
        sl = slice(m * NQ, (m + 1) * NQ)
        im = dict(common)
        im["xq"] = f(x[:, sl])
        im["spt"] = f(spT[:, sl])
        in_maps.append(im)
    has_bv = bool(np.any(bv != 0))
    has_b3 = bool(np.any(b3 != 0))
    return in_maps, has_bv, has_b3


def _run(inputs, trace=False):
    from concourse.bass_utils import run_bass_kernel_spmd
    in_maps, has_bv, has_b3 = _prep_inputs(inputs)
    key = (has_bv, has_b3)
    if key not in _CACHE:
        _CACHE[key] = _build(has_bv, has_b3)
    nc = _CACHE[key]
    res = run_bass_kernel_spmd(nc, in_maps, core_ids=list(range(NCORES)),
                               trace=trace)
    out = np.concatenate([res.results[m]["out"] for m in range(NCORES)],
                         axis=1)[None]
    return np.ascontiguousarray(out.astype(np.float32)), res


def kernel(**inputs):
    out, _ = _run(inputs, trace=False)
    return out
